# revision 1
# baseline (speedup 1.0000x reference)
"""FNO-style ComplexLinear spectral conv on 8 trn2 NeuronCores.

Pipeline: 5 SPMD bass launches of dense matmul stages; host numpy reshards
between launches. Active low-freq modes live in a centered 68x68 box.
  A: H-DFT (192->68), sharded by w-band
  B: W-DFT (192->68), sharded by hh-band (padded 72)
  C: C-DFT + per-mode (c->f) mix + F-IDFT, sharded by mode-band
  D: W-IDFT (68->192), sharded by hh-band
  E: H-IDFT (68->192, real out), sharded by w-band
"""

import numpy as np
import ml_dtypes

import concourse.bacc as bacc
import concourse.bass as bass
import concourse.mybir as mybir
import concourse.tile as tile
from concourse.bass_utils import run_bass_kernel_spmd

H, W, C, F = 192, 192, 16, 32
B = 8
NCORES = 8
THRES = 0.25
R0, R1 = 62, 130          # centered active rows [62, 129]
NB = 68                   # box edge
NMODES = NB * NB          # 4624
MB = NMODES // NCORES     # 578 modes per core
HHP = 72                  # hh padded to 72 -> 9 rows/core
F32 = mybir.dt.float32
BF16 = mybir.dt.bfloat16
F32R = mybir.dt.float32r
MMDT = F32R

_f68 = (np.arange(R0, R1) - 96) % 192          # active freq indices
LAST_EXEC_NS = []          # per-launch hw exec ns (filled when trace on)
TRACE = False


def _mask2d():
    ii, jj = np.meshgrid(np.arange(H), np.arange(W), indexing="ij")
    d = np.sqrt((ii - (H - 1) / 2.0) ** 2 + (jj - (W - 1) / 2.0) ** 2)
    return (d / d.max()) < THRES


def _mats():
    h = np.arange(H)[:, None]
    ang = 2 * np.pi * h * _f68[None, :] / H          # [192, 68]
    Ar, Ai = np.cos(ang), -np.sin(ang)               # fwd DFT lhsT [h, hh]
    angT = ang.T                                     # [68, 192]
    IHr, IHi = np.cos(angT) / H, np.sin(angT) / H    # inv DFT lhsT [hh, h]
    c0 = np.arange(C)[:, None]
    angc = 2 * np.pi * c0 * np.arange(C)[None, :] / C
    Cr, Ci = np.cos(angc), -np.sin(angc)             # [c0, c]
    f = np.arange(F)[:, None]
    angf = 2 * np.pi * f * np.arange(F)[None, :] / F
    Fr, Fi = np.cos(angf) / F, np.sin(angf) / F      # inv over F [f, ff]
    return (Ar.astype(np.float32), Ai.astype(np.float32),
            IHr.astype(np.float32), IHi.astype(np.float32),
            Cr.astype(np.float32), Ci.astype(np.float32),
            Fr.astype(np.float32), Fi.astype(np.float32))


AR, AI, IHR, IHI, CR, CI, FR, FI = _mats()


def _stack_complex(Mr, Mi):
    """lhsT [2K, 2P]: rows (r-block; i-block), cols (r-block; i-block)."""
    K, P = Mr.shape
    S = np.zeros((2 * K, 2 * P), np.float32)
    S[:K, :P] = Mr
    S[K:, :P] = -Mi
    S[:K, P:] = Mi
    S[K:, P:] = Mr
    return S


CSTACK = _stack_complex(CR, CI)      # [32, 32]
FSTACK = _stack_complex(FR, FI)      # [64, 64]


def _dft_stage(nc, tc, Xr, Xi, Mr, Mi, out_r, out_i, kdim, pout, nfree,
               real_in=False, real_out=False, mmdt=F32):
    """out = M^T @ X (complex), M given as lhsT [kdim, pout] r/i parts.

    Xr/Xi, out_r/out_i are DRAM APs [kdim, nfree] / [pout, nfree].
    """
    kch = [(s, min(128, kdim - s)) for s in range(0, kdim, 128)]
    pch = [(s, min(128, pout - s)) for s in range(0, pout, 128)]
    nch = [(s, min(512, nfree - s)) for s in range(0, nfree, 512)]
    with (
        tc.tile_pool(name="mat", bufs=1) as matp,
        tc.tile_pool(name="xin", bufs=1) as xinp,
        tc.tile_pool(name="ps", bufs=4, space="PSUM") as psp,
        tc.tile_pool(name="so", bufs=4) as sop,
    ):
        mats = {}
        for (ks, kn) in kch:
            for (ps, pn) in pch:
                for nm, M in (("r", Mr), ("i", Mi), ("ni", -Mi)):
                    if real_in and nm == "ni":
                        continue
                    t = matp.tile([kn, pn], mmdt, name=f"m{nm}{ks}{ps}")
                    nc.sync.dma_start(t[:, :], _const_ap(nc, M[ks:ks + kn, ps:ps + pn], f"M{nm}_{ks}_{ps}", dt=mmdt))
                    mats[(nm, ks, ps)] = t
        xt = {}
        for (ks, kn) in kch:
            for nm, Xa in (("r", Xr), ("i", Xi)):
                if real_in and nm == "i":
                    continue
                t = xinp.tile([kn, nfree], mmdt, name=f"x{nm}{ks}")
                nc.sync.dma_start(t[:, :], Xa[ks:ks + kn, :])
                xt[(nm, ks)] = t
        outs = [("r", out_r)] if real_out else [("r", out_r), ("i", out_i)]
        for (ps, pn) in pch:
            for (ns, nn) in nch:
                for oi, (onm, oap) in enumerate(outs):
                    acc = psp.tile([pn, nn], F32, name=f"ps{ps}{ns}{onm}", tag="ps")
                    first = True
                    for (ks, kn) in kch:
                        if onm == "r":
                            pairs = [("r", "r")] if real_in else [("r", "r"), ("ni", "i")]
                        else:
                            pairs = [("i", "r")] if real_in else [("i", "r"), ("r", "i")]
                        for mnm, xnm in pairs:
                            last = (ks == kch[-1][0]) and (mnm == pairs[-1][0])
                            nc.tensor.matmul(
                                acc[:, :], mats[(mnm, ks, ps)][:, :],
                                xt[(xnm, ks)][:, ns:ns + nn],
                                start=first, stop=last)
                            first = False
                    ot = sop.tile([pn, nn], F32, name=f"o{ps}{ns}{onm}", tag="so")
                    nc.vector.tensor_copy(ot[:, :], acc[:, :])
                    nc.sync.dma_start(oap[ps:ps + pn, ns:ns + nn], ot[:, :])


_CONST_CACHE = {}


def _const_ap(nc, arr, name, dt=None):
    """DRAM ExternalInput holding a host constant; registered per-launch."""
    key = (id(nc), name)
    if key not in _CONST_CACHE:
        a = np.ascontiguousarray(arr)
        t = nc.dram_tensor(name, list(a.shape), dt or mybir.dt.from_np(a.dtype),
                           kind="ExternalInput").ap()
        nc.ant_const_inputs[name] = a        # stashed; collected into in_maps
        _CONST_CACHE[key] = t
    return _CONST_CACHE[key]


def _new_nc():
    nc = bacc.Bacc("TRN2", target_bir_lowering=False, debug=False,
                   enable_asserts=False, num_devices=NCORES)
    nc.ant_const_inputs = {}
    return nc


def _run(nc, per_core_inputs, label):
    in_maps = []
    for c in range(NCORES):
        m = dict(per_core_inputs[c])
        m.update(nc.ant_const_inputs)
        in_maps.append(m)
    import time as _t
    t0 = _t.time()
    res = run_bass_kernel_spmd(nc, in_maps, core_ids=list(range(NCORES)),
                               trace=False)
    dt_ns = int((_t.time() - t0) * 1e9)
    LAST_EXEC_NS.append((label, res.exec_time_ns or dt_ns))
    return res.results


def _build_stage_nc(label, kdim, pout, nfree, real_in, real_out, mmdt=F32):
    nc = _new_nc()
    Xr = nc.dram_tensor("xr", [kdim, nfree], mmdt, kind="ExternalInput").ap()
    Xi = None
    if not real_in:
        Xi = nc.dram_tensor("xi", [kdim, nfree], mmdt, kind="ExternalInput").ap()
    out_r = nc.dram_tensor("outr", [pout, nfree], F32, kind="ExternalOutput").ap()
    out_i = None
    if not real_out:
        out_i = nc.dram_tensor("outi", [pout, nfree], F32, kind="ExternalOutput").ap()
    return nc, Xr, Xi, out_r, out_i


def _stage_launch(label, Mr, Mi, xr_list, xi_list, real_in, real_out, mmdt=F32):
    kdim, pout = Mr.shape
    nfree = xr_list[0].shape[1]
    nc, Xr, Xi, out_r, out_i = _build_stage_nc(label, kdim, pout, nfree,
                                               real_in, real_out, mmdt=mmdt)
    with tile.TileContext(nc) as tc:
        _dft_stage(nc, tc, Xr, Xi, Mr, Mi, out_r, out_i, kdim, pout, nfree,
                   real_in=real_in, real_out=real_out, mmdt=mmdt)
    nc.compile()
    per_core = []
    for c in range(NCORES):
        m = {"xr": np.ascontiguousarray(xr_list[c])}
        if not real_in:
            m["xi"] = np.ascontiguousarray(xi_list[c])
        per_core.append(m)
    res = _run(nc, per_core, label)
    outr = [r["outr"] for r in res]
    outi = [r.get("outi") for r in res]
    return outr, outi


def _mix_launch(xc_list, kr_list, ki_list):
    """Per core: C-DFT (stacked complex), per-mode mix, F-IDFT.

    xc: [32, MB*8] f32, rows = (c0 r-block 16; i-block 16), cols (m, b).
    kr/ki: [32, MB*32] bf16 pass tiles, rows (c 16; c 16), cols (m, f):
      kr rows0-15 = Kr, rows16-31 = -Ki ; ki rows0-15 = Ki, rows16-31 = Kr.
    out: [64, MB*8] f32 rows (ff r-block 32; i-block 32).
    """
    nfree = MB * B
    nc = _new_nc()
    XC = nc.dram_tensor("xc", [32, nfree], F32, kind="ExternalInput").ap()
    KR = nc.dram_tensor("kr", [32, MB * F], BF16, kind="ExternalInput").ap()
    KI = nc.dram_tensor("ki", [32, MB * F], BF16, kind="ExternalInput").ap()
    OUT = nc.dram_tensor("outr", [64, nfree], F32, kind="ExternalOutput").ap()
    with tile.TileContext(nc) as tc:
        with (
            tc.tile_pool(name="cst", bufs=1) as cstp,
            tc.tile_pool(name="xin", bufs=1) as xinp,
            tc.tile_pool(name="ps", bufs=4, space="PSUM") as psp,
            tc.tile_pool(name="mx", bufs=1) as mxp,
            tc.tile_pool(name="so", bufs=4) as sop,
        ):
            cstk = cstp.tile([32, 32], F32, name="cstk")
            nc.sync.dma_start(cstk[:, :], _const_ap(nc, CSTACK, "CSTACK"))
            fstk = cstp.tile([64, 64], BF16, name="fstk")
            nc.sync.dma_start(fstk[:, :], _const_ap(nc, FSTACK.astype(ml_dtypes.bfloat16), "FSTACK"))
            xc = xinp.tile([32, nfree], F32, name="xc")
            nc.sync.dma_start(xc[:, :], XC[:, :])
            krt = xinp.tile([32, MB * F], BF16, name="krt")
            nc.sync.dma_start(krt[:, :], KR[:, :])
            kit = xinp.tile([32, MB * F], BF16, name="kit")
            nc.sync.dma_start(kit[:, :], KI[:, :])
            # 1) C-DFT: xm = CSTACK^T @ xc  (bf16 out for the mix)
            xm = mxp.tile([32, nfree], BF16, name="xm")
            for ns in range(0, nfree, 512):
                nn = min(512, nfree - ns)
                ps = psp.tile([32, nn], F32, name="cps", tag="ps", bufs=2)
                nc.tensor.matmul(ps[:, :], cstk[:, :], xc[:, ns:ns + nn],
                                 start=True, stop=True)
                nc.vector.tensor_copy(xm[:, ns:ns + nn], ps[:, :])
            # 2) per-mode mix -> mixed_sb [64, nfree] bf16
            mixed = mxp.tile([64, nfree], BF16, name="mixed")
            for t0 in range(0, MB, 64):
                tn = min(64, MB - t0)
                pr = psp.tile([32, tn * B], F32, name="mixr", tag="psr", bufs=2)
                pi = psp.tile([32, tn * B], F32, name="mixi", tag="psi", bufs=2)
                for m in range(tn):
                    mm = t0 + m
                    nc.tensor.matmul(pr[:, m * B:(m + 1) * B],
                                     krt[:, mm * F:(mm + 1) * F],
                                     xm[:, mm * B:(mm + 1) * B],
                                     start=True, stop=True)
                    nc.tensor.matmul(pi[:, m * B:(m + 1) * B],
                                     kit[:, mm * F:(mm + 1) * F],
                                     xm[:, mm * B:(mm + 1) * B],
                                     start=True, stop=True)
                nc.vector.tensor_copy(mixed[0:32, t0 * B:(t0 + tn) * B], pr[:, :])
                nc.vector.tensor_copy(mixed[32:64, t0 * B:(t0 + tn) * B], pi[:, :])
            # 3) F-IDFT: out = FSTACK^T @ mixed
            for ns in range(0, nfree, 512):
                nn = min(512, nfree - ns)
                ps2 = psp.tile([64, nn], F32, name="fps", tag="ps", bufs=2)
                nc.tensor.matmul(ps2[:, :], fstk[:, :], mixed[:, ns:ns + nn],
                                 start=True, stop=True)
                ot = sop.tile([64, nn], F32, name="fo", tag="so")
                nc.vector.tensor_copy(ot[:, :], ps2[:, :])
                nc.sync.dma_start(OUT[:, ns:ns + nn], ot[:, :])
    nc.compile()
    per_core = [{"xc": np.ascontiguousarray(xc_list[c]),
                 "kr": np.ascontiguousarray(kr_list[c]),
                 "ki": np.ascontiguousarray(ki_list[c])} for c in range(NCORES)]
    res = _run(nc, per_core, "C-mix")
    return [r["outr"] for r in res]


def kernel(inputs, real_kernel, imag_kernel):
    global _CONST_CACHE
    _CONST_CACHE = {}
    LAST_EXEC_NS.clear()
    x = np.asarray(inputs, np.float32)            # (B, H, W, C)
    WB = W // NCORES                              # 24
    # ---- A: H-DFT, shard by w-band. in [192h, (b, wb, c)]
    xr_list = [np.ascontiguousarray(x[:, :, c * WB:(c + 1) * WB, :]
                                    .transpose(1, 0, 2, 3).reshape(H, B * WB * C))
               for c in range(NCORES)]
    ar, ai = _stage_launch("A-hdft", AR, AI, xr_list, None, True, False, mmdt=MMDT)
    # reassemble X1[hh(68), b, w, c] r/i
    X1r = np.zeros((NB, B, W, C), np.float32)
    X1i = np.zeros((NB, B, W, C), np.float32)
    for c in range(NCORES):
        X1r[:, :, c * WB:(c + 1) * WB, :] = ar[c].reshape(NB, B, WB, C)
        X1i[:, :, c * WB:(c + 1) * WB, :] = ai[c].reshape(NB, B, WB, C)
    # ---- B: W-DFT, shard by hh-band (pad 68->72, 9/core). in [192w, (b, hb, c)]
    HB = HHP // NCORES                            # 9
    X1rp = np.zeros((HHP, B, W, C), np.float32); X1rp[:NB] = X1r
    X1ip = np.zeros((HHP, B, W, C), np.float32); X1ip[:NB] = X1i
    br_in = [np.ascontiguousarray(X1rp[c * HB:(c + 1) * HB].transpose(2, 1, 0, 3)
                                  .reshape(W, B * HB * C)) for c in range(NCORES)]
    bi_in = [np.ascontiguousarray(X1ip[c * HB:(c + 1) * HB].transpose(2, 1, 0, 3)
                                  .reshape(W, B * HB * C)) for c in range(NCORES)]
    br, bi = _stage_launch("B-wdft", AR, AI, br_in, bi_in, False, False, mmdt=MMDT)
    # X2[hh68, ww68, b, c] complex
    X2r = np.zeros((HHP, NB, B, C), np.float32)
    X2i = np.zeros((HHP, NB, B, C), np.float32)
    for c in range(NCORES):
        X2r[c * HB:(c + 1) * HB] = br[c].reshape(NB, B, HB, C).transpose(2, 0, 1, 3)
        X2i[c * HB:(c + 1) * HB] = bi[c].reshape(NB, B, HB, C).transpose(2, 0, 1, 3)
    X2r = X2r[:NB]; X2i = X2i[:NB]
    # ---- C: mode-sharded mix. xc [32=(c r;i), (m, b)]
    Xm = np.concatenate([X2r, X2i], axis=-1)      # [68, 68, B, 32]
    Xm = Xm.reshape(NMODES, B, 2 * C).transpose(0, 2, 1)  # [m, 32, b]
    xc_list = [np.ascontiguousarray(Xm[c * MB:(c + 1) * MB].transpose(1, 0, 2)
                                    .reshape(2 * C, MB * B)) for c in range(NCORES)]
    # K scatter disk->box
    mask = _mask2d()
    act = np.argwhere(mask)                       # row-major sorted
    bidx = (act[:, 0] - R0) * NB + (act[:, 1] - R0)
    Kr = np.zeros((NMODES, C, F), np.float32)
    Ki = np.zeros((NMODES, C, F), np.float32)
    Kr[bidx] = np.asarray(real_kernel, np.float32).reshape(-1, C, F)
    Ki[bidx] = np.asarray(imag_kernel, np.float32).reshape(-1, C, F)
    kr_pass = np.concatenate([Kr, -Ki], axis=1)   # [m, 32, F]
    ki_pass = np.concatenate([Ki, Kr], axis=1)
    kr_list = [np.ascontiguousarray(kr_pass[c * MB:(c + 1) * MB].transpose(1, 0, 2)
                                    .reshape(2 * C, MB * F)).astype(ml_dtypes.bfloat16)
               for c in range(NCORES)]
    ki_list = [np.ascontiguousarray(ki_pass[c * MB:(c + 1) * MB].transpose(1, 0, 2)
                                    .reshape(2 * C, MB * F)).astype(ml_dtypes.bfloat16)
               for c in range(NCORES)]
    mo = _mix_launch(xc_list, kr_list, ki_list)
    # mixed2[m, ff, b] complex  (rows 0-31 real, 32-63 imag)
    M2r = np.zeros((NMODES, F, B), np.float32)
    M2i = np.zeros((NMODES, F, B), np.float32)
    for c in range(NCORES):
        o = mo[c].reshape(64, MB, B)
        M2r[c * MB:(c + 1) * MB] = o[:32].transpose(1, 0, 2)
        M2i[c * MB:(c + 1) * MB] = o[32:].transpose(1, 0, 2)
    M2r = M2r.reshape(NB, NB, F, B)               # [hh, ww, ff, b]
    M2i = M2i.reshape(NB, NB, F, B)
    # ---- D: W-IDFT, shard by hh-band. in [ww68, (b, hb9, ff)]
    M2rp = np.zeros((HHP, NB, F, B), np.float32); M2rp[:NB] = M2r.transpose(0, 1, 2, 3)
    M2ip = np.zeros((HHP, NB, F, B), np.float32); M2ip[:NB] = M2i
    dr_in = [np.ascontiguousarray(M2rp[c * HB:(c + 1) * HB].transpose(1, 3, 0, 2)
                                  .reshape(NB, B * HB * F)) for c in range(NCORES)]
    di_in = [np.ascontiguousarray(M2ip[c * HB:(c + 1) * HB].transpose(1, 3, 0, 2)
                                  .reshape(NB, B * HB * F)) for c in range(NCORES)]
    IWr = IHR * H / W if H != W else IHR          # same matrix
    dr, di = _stage_launch("D-widft", IHR, IHI, dr_in, di_in, False, False, mmdt=MMDT)
    # X4[hh68, w192, ff, b]
    X4r = np.zeros((HHP, W, F, B), np.float32)
    X4i = np.zeros((HHP, W, F, B), np.float32)
    for c in range(NCORES):
        X4r[c * HB:(c + 1) * HB] = dr[c].reshape(W, B, HB, F).transpose(2, 0, 3, 1)
        X4i[c * HB:(c + 1) * HB] = di[c].reshape(W, B, HB, F).transpose(2, 0, 3, 1)
    X4r = X4r[:NB]; X4i = X4i[:NB]
    # ---- E: H-IDFT real out, shard by w-band. in [hh68, (b, wb24, ff)]
    er_in = [np.ascontiguousarray(X4r[:, c * WB:(c + 1) * WB].transpose(0, 3, 1, 2)
                                  .reshape(NB, B * WB * F)) for c in range(NCORES)]
    ei_in = [np.ascontiguousarray(X4i[:, c * WB:(c + 1) * WB].transpose(0, 3, 1, 2)
                                  .reshape(NB, B * WB * F)) for c in range(NCORES)]
    er, _ = _stage_launch("E-hidft", IHR, IHI, er_in, ei_in, False, True, mmdt=MMDT)
    out = np.zeros((B, H, W, F), np.float32)
    for c in range(NCORES):
        out[:, :, c * WB:(c + 1) * WB, :] = er[c].reshape(H, B, WB, F).transpose(1, 0, 2, 3)
    return out

